# revision 1
# baseline (speedup 1.0000x reference)
"""DiffAttention TRN2 Bass kernel, v2 (query-on-partitions PV).

Problem: nn_DiffAttention_15977278341927
  B=2, N=2048, DIM=1024, 16 heads of dim 64 -> 8 effective head-pairs.
  out = ((softmax(Q1K1^T) - lam*softmax(Q2K2^T)) @ V) -> headwise RMSNorm
        -> *(1-lam_init) -> concat heads -> @ Wo + bo

Sharding (8 cores): batch (2) x head-pair groups (4 groups of 2 pairs).
Core c: batch c//4, pairs {2*(c%4), 2*(c%4)+1}.  QKV col-sharded, Wo
row-sharded (pre-scaled by g*(1-lam_init) on host), partials summed on host.

v2 dataflow:
  QT,KT [d, n] as before (S^T tiles keys-on-partitions), but V^T [token, vd]
  produced directly by swapping matmul operands (xt tile as lhsT).  PV runs
  q-on-partitions: U[q, vd] += ex[k, q]^T @ V[k, vd] via 128-col matmuls, and
  softmax denominators are 1-col matmuls reusing the same stationary ex.
  All per-token scalars (f = lam*s1/s2, rms) are per-partition [128,x] ops:
  no gpsimd broadcasts, no O(N^2) vector work, no ones-row denominators.
  rsqrt via Ln/Exp (same activation table as the softmax Exp).
"""

import os
import sys

sys.path.insert(0, "/opt/trn_rl_repo")

import ml_dtypes
import numpy as np

import concourse.bacc as bacc
import concourse.bass as bass
import concourse.mybir as mybir
import concourse.tile as tile
from concourse.masks import make_identity

# --- custom fused DVE exp: ex = (((x*b3 + b2)*x + b1)*x + b0)^16 ---------
# cubic fit of e^{x/16} on [-4.5, 4.5]; rel err of the ^16 composite is
# <1.3e-3 (f32 interior, f16 output).  Offloads part of the softmax exp
# from the Activation engine (the phase-2 bottleneck) to the DVE.
from concourse.dve_spec import Spec as _DveSpec, Src0 as _Src0, C0 as _C0,     C1 as _C1, C2 as _C2, sq as _dve_sq, lower as _dve_lower
from concourse.dve_uop import DveOpSpec as _DveOpSpec
import concourse.dve_ops as _dve_ops

EXP_B = (0.9999775855657377, 0.062499222135353624,
         0.001964185205139091, 4.086924886446811e-05)


def _register_exp_ops():
    have = {o.name: o for o in _dve_ops.OPS}
    if "ANT_EXP_P3" in have:
        return have["ANT_EXP_P3"], have["ANT_POW16"]
    import numpy as _np

    specA = _DveSpec(
        body=((_Src0 * _C0 + _C1) * _Src0 + _C2) * _Src0,
        reference=lambda in0, in1, s0, s1, imm2: (
            ((in0.astype(_np.float32) * s0 + s1) * in0 + imm2) * in0
        ).astype(_np.float32),
    )

    def _refB(in0, in1, s0, s1, imm2):
        t = in0.astype(_np.float32) + s0
        for _ in range(4):
            t = (t * t).astype(_np.float32)
        return t

    specB = _DveSpec(body=_dve_sq(_dve_sq(_dve_sq(_dve_sq(_Src0 + _C0)))),
                     reference=_refB)
    out = []
    for name, spec in (("ANT_EXP_P3", specA), ("ANT_POW16", specB)):
        opcode = _dve_ops._CUSTOM_DVE_ROW_BASE + len(_dve_ops.OPS)
        shas = {
            ver: _DveOpSpec(
                name=name, opcode=opcode, uops=_dve_lower(spec, ver=ver),
                rd1_en=False,
            ).sha(ver)
            for ver in ("v3", "v4")
        }
        op = _dve_ops.DveOp(name, spec, subdim=False, uops_sha=shas)
        _dve_ops.OPS.append(op)
        _dve_ops._SUB_OPCODE_FOR_NAME[name] = opcode
        _dve_ops.CUSTOM_DVE_SPECS[name] = spec
        out.append(op)
    return out[0], out[1]


_EXP_P3_OP, _POW16_OP = _register_exp_ops()

B, N, DIM = 2, 2048, 1024
EFF = 8
HEAD_DIM = 64
D2 = 2 * HEAD_DIM  # 128: one head-pair's q/k cols and one v head's dims
LAMBDA_INIT = 0.8
EPS = 1e-5

P = 128
CH = 512  # S^T tile q-chunk
NCH = N // CH  # 4
MT = N // P  # 16 key tiles
KT_D = DIM // P  # 8 contraction tiles over DIM
PAIRS = 2
CORES = 8

F32 = mybir.dt.float32
F16 = mybir.dt.float16
DEBUG = bool(int(os.environ.get("KERNEL_DEBUG", "0")))
ABL = os.environ.get("KERNEL_ABL", "")


def _emit(tc, t, iters=1):
    from contextlib import ExitStack
    import contextlib

    nc = tc.nc

    ctx = ExitStack()
    with ctx:
        singles = ctx.enter_context(tc.tile_pool(name="singles", bufs=1))
        big = ctx.enter_context(tc.tile_pool(name="big", bufs=2))
        expp = ctx.enter_context(tc.tile_pool(name="expp", bufs=6))
        udp = ctx.enter_context(tc.tile_pool(name="udp", bufs=2))
        onp = ctx.enter_context(tc.tile_pool(name="onp", bufs=2))
        sm = ctx.enter_context(tc.tile_pool(name="sm", bufs=4))
        ypool = ctx.enter_context(tc.tile_pool(name="ypool", bufs=4))
        dvp = ctx.enter_context(tc.tile_pool(name="dvp", bufs=2))
        pa = ctx.enter_context(tc.tile_pool(name="pa", bufs=2, space="PSUM"))
        pu = ctx.enter_context(tc.tile_pool(name="pu", bufs=1, space="PSUM"))
        pb = ctx.enter_context(tc.tile_pool(name="pb", bufs=2, space="PSUM"))

        # ---- loads / constants ----
        xt_sb = singles.tile([P, KT_D, N], F16)
        nc.sync.dma_start(xt_sb, t["xt"].rearrange("(k p) n -> p k n", p=P))
        w_sb = {}
        for w in ("wq", "wk", "wv"):
            w_sb[w] = singles.tile([P, KT_D, PAIRS * D2], F16, name=f"w_{w}")
            nc.sync.dma_start(w_sb[w], t[w].rearrange("(k p) c -> p k c", p=P))
        wo_sb = singles.tile([P, PAIRS, DIM], F16)
        nc.sync.dma_start(wo_sb, t["wo"].rearrange("(u p) c -> p u c", p=P))
        params_sb = singles.tile([P, 8], F32)
        nc.gpsimd.dma_start(params_sb, t["params"])
        bvr_sb = singles.tile([1, PAIRS * D2], F16)
        nc.gpsimd.dma_start(bvr_sb, t["bvr"])
        # bv broadcast across partitions once at setup; added during the
        # V^T psum eviction (varies along the free dim there)
        bvb_sb = singles.tile([P, PAIRS * D2], F16)
        nc.gpsimd.partition_broadcast(bvb_sb, bvr_sb, channels=P)
        ones_row = singles.tile([1, P], F16)
        nc.vector.memset(ones_row, 1.0)
        ones_col = singles.tile([P, 1], F16)
        nc.vector.memset(ones_col, 1.0)
        ident = singles.tile([P, P], F16)
        make_identity(nc, ident)
        # transposed normalized outputs, [vd, pair, n]
        outnT = singles.tile([P, PAIRS, N], F16)
        if ABL in ("p1", "sonly", "nopv", "noden", "noud"):
            nc.vector.memset(outnT, 0.001)

        env = dict(locals())
        loop_cm = (
            tc.For_i(0, iters, 1, hint_engines=(mybir.EngineType.PE,))
            if iters > 1
            else contextlib.nullcontext()
        )
        with loop_cm:
            _emit_body(tc, t, env)


def _emit_body(tc, t, env):
    nc = tc.nc
    Act = mybir.ActivationFunctionType
    Alu = mybir.AluOpType
    big = env["big"]; expp = env["expp"]; udp = env["udp"]; onp = env["onp"]
    sm = env["sm"]; ypool = env["ypool"]
    pa = env["pa"]; pu = env["pu"]; pb = env["pb"]
    xt_sb = env["xt_sb"]; w_sb = env["w_sb"]; wo_sb = env["wo_sb"]
    params_sb = env["params_sb"]; bvr_sb = env["bvr_sb"]
    ones_row = env["ones_row"]; ones_col = env["ones_col"]; ident = env["ident"]
    outnT = env["outnT"]

    # ---- phase 1 (both pairs): QT,KT [d-pair, n]; V^T [token, vd] ----
    QTs, KTs, Vs = [], [], []
    for u in range(PAIRS):
        usl = slice(u * D2, (u + 1) * D2)
        QT = big.tile([P, N], F16, tag="QT")
        KTt = big.tile([P, N], F16, tag="KTt")
        V_sb = big.tile([P, MT, P], F16, tag="V")
        QTs.append(QT); KTs.append(KTt); Vs.append(V_sb)
        for nch in range(NCH):
            sl = slice(nch * CH, (nch + 1) * CH)
            ps = pa.tile([P, 2, CH], F32, tag="pa")
            for kt in range(KT_D):
                nc.tensor.matmul(
                    ps[:, 0, :],
                    lhsT=w_sb["wq"][:, kt, usl],
                    rhs=xt_sb[:, kt, sl],
                    start=(kt == 0),
                    stop=(kt == KT_D - 1),
                )
            for kt in range(KT_D):
                nc.tensor.matmul(
                    ps[:, 1, :],
                    lhsT=w_sb["wk"][:, kt, usl],
                    rhs=xt_sb[:, kt, sl],
                    start=(kt == 0),
                    stop=(kt == KT_D - 1),
                )
            nc.vector.tensor_scalar_add(QT[:, sl], ps[:, 0, :], params_sb[:, u : u + 1])
            nc.vector.tensor_scalar_add(
                KTt[:, sl], ps[:, 1, :], params_sb[:, 2 + u : 3 + u]
            )
            # V^T for the 4 token-tiles of this chunk
            for i in range(CH // P):
                tt = nch * (CH // P) + i
                tsl = slice(tt * P, (tt + 1) * P)
                vt = pb.tile([P, P], F32, tag="pb", name="vt")
                for kt in range(KT_D):
                    nc.tensor.matmul(
                        vt,
                        lhsT=xt_sb[:, kt, tsl],
                        rhs=w_sb["wv"][:, kt, usl],
                        start=(kt == 0),
                        stop=(kt == KT_D - 1),
                    )
                nc.vector.tensor_tensor(
                    V_sb[:, tt, :], vt, env["bvb_sb"][:, usl], Alu.add
                )

    if DEBUG:
        nc.gpsimd.dma_start(t["dbg_qt"], QTs[0])
        nc.gpsimd.dma_start(t["dbg_kt"], KTs[0])
        nc.gpsimd.dma_start(t["dbg_v"], Vs[0].rearrange("p a b -> p (a b)"))

    # ---- phase 2 (both pairs): S^T -> exp -> U[q,vd] + denominators ----
    for u in range(0 if ABL in ('p1', 'p1o') else PAIRS):
        QT, KTt, V_sb = QTs[u], KTs[u], Vs[u]
        uds = []
        msq = sm.tile([P, MT], F32, tag="msq", bufs=2)
        epsq_all = sm.tile([P, MT], F32, tag="epsq_all", bufs=2)
        for nch in range(NCH):
            sl = slice(nch * CH, (nch + 1) * CH)
            Up = pu.tile([P, 8, P], F32, tag="pu")
            den = pb.tile([P, 8, 1], F32, tag="pb", name="den")

            exq = {}

            def _issue_exp(mt, sp):
                if ABL == 'sonly':
                    return
                ex = expp.tile([P, 2, CH], F16, tag="ex", name="ex")
                if mt % 4 == 2 and ABL != 'noexpdve':
                    # DVE fused-poly exp: two custom ops
                    tA = env["dvp"].tile([P, 2, CH], F32, tag="tA", name="tA")
                    nc.vector._custom_dve(
                        _EXP_P3_OP, out=tA, in0=sp,
                        s0=EXP_B[3], s1=EXP_B[2], imm2=EXP_B[1],
                    )
                    nc.vector._custom_dve(
                        _POW16_OP, out=ex, in0=tA, s0=EXP_B[0],
                    )
                else:
                    nc.scalar.activation(ex, sp, Act.Exp)
                exq[mt] = ex

            def _issue_pv(mt):
                if ABL == 'sonly':
                    return
                ex = exq.pop(mt)
                # banks hold accumulators {0-3} and {4-7}: only the first
                # matmul into each bank starts, only the last stops
                for h in range(2):
                    for j in range(CH // P):
                        lh = ex[:, h, j * P : (j + 1) * P]
                        k = 4 * h + j
                        nc.tensor.matmul(
                            Up[:, k, :],
                            lhsT=lh,
                            rhs=V_sb[:, mt, :],
                            start=(mt == 0 and j == 0),
                            stop=(mt == MT - 1 and j == 3),
                        )
                        if ABL != 'noden':
                            nc.tensor.matmul(
                                den[:, k, :],
                                lhsT=lh,
                                rhs=ones_col,
                                start=(mt == 0 and k == 0),
                                stop=(mt == MT - 1 and k == 7),
                            )

            for mt in range(MT):
                msl = slice(mt * P, (mt + 1) * P)
                sp = pa.tile([P, 2, CH], F32, tag="pa")
                nc.tensor.matmul(
                    sp[:, 0, :], lhsT=KTt[0:64, msl], rhs=QT[0:64, sl]
                )
                nc.tensor.matmul(
                    sp[:, 1, :], lhsT=KTt[64:128, msl], rhs=QT[64:128, sl]
                )
                _issue_exp(mt, sp)
                if mt >= 2:
                    _issue_pv(mt - 2)
            _issue_pv(MT - 2)
            _issue_pv(MT - 1)

            # ---- post-processing: all per-partition [128, 4] ops ----
            # order: Up readers (ud) first so the single-buffered pu frees
            # for the next chunk's accumulation as early as possible.
            if ABL in ('noud', 'nopv', 'noden', 'sonly'):
                continue
            # DVE may read only ONE non-scalar input from PSUM per op:
            # stage s1 and U1 to SBUF first.
            s1c = sm.tile([P, 4], F32, tag="s1c")
            nc.vector.tensor_copy(s1c, den[:, 0:4, 0])
            r2 = sm.tile([P, 4], F32, tag="r2")
            nc.vector.reciprocal(r2, den[:, 4:8, 0])
            # f = -lam * s1 / s2   (params[:,7] = -lam)
            f = sm.tile([P, 4], F32, tag="f")
            nc.vector.scalar_tensor_tensor(
                out=f, in0=s1c, scalar=params_sb[:, 7:8],
                in1=r2, op0=Alu.mult, op1=Alu.mult,
            )
            u1c = udp.tile([P, 4, P], F32, tag="u1c")
            nc.vector.tensor_copy(u1c, Up[:, 0:4, :])
            ud = udp.tile([P, 4, P], F16, tag="ud", bufs=5)
            uds.append(ud)
            for j in range(4):
                # ud = U1 + f*U2  (f negated)
                nc.vector.scalar_tensor_tensor(
                    out=ud[:, j, :], in0=Up[:, 4 + j, :],
                    scalar=f[:, j : j + 1], in1=u1c[:, j, :],
                    op0=Alu.mult, op1=Alu.add,
                )
            # epsq = (P*eps) * s1^2  (added to ssq per pair, before Ln)
            nc.vector.scalar_tensor_tensor(
                out=epsq_all[:, nch * 4 : nch * 4 + 4], in0=s1c,
                scalar=float(P) * EPS,
                in1=s1c, op0=Alu.mult, op1=Alu.mult,
            )
            # ssq: square (f16 fast) + free-axis reduce; tensor_tensor_reduce
            # faults TRN2 here, so use the two-op form
            sqscr = udp.tile([P, 4, P], F32, tag="sq")
            nc.vector.tensor_mul(sqscr, ud, ud)
            for j in range(4):
                nc.vector.reduce_sum(
                    out=msq[:, nch * 4 + j : nch * 4 + j + 1],
                    in_=sqscr[:, j, :],
                    axis=mybir.AxisListType.X,
                )

        if ABL in ('noud', 'nopv', 'noden', 'sonly'):
            continue
        # rinv = rsqrt((ssq + P*eps*s1^2) / P) = exp(-0.5 * ln(msqe/P))
        # batched per pair: one Ln + one Exp, minimizing act-table switches
        nc.vector.tensor_add(msq, msq, epsq_all)
        lnm = sm.tile([P, MT], F32, tag="lnm", bufs=2)
        nc.scalar.activation(lnm, msq, Act.Ln, scale=1.0 / P)
        rinv = sm.tile([P, MT], F32, tag="rinv", bufs=2)
        nc.scalar.activation(rinv, lnm, Act.Exp, scale=-0.5)
        if DEBUG and u == 0:
            nc.gpsimd.dma_start(t["dbg_rinv"], rinv)
        for nch in range(NCH):
            sl = slice(nch * CH, (nch + 1) * CH)
            onx = onp.tile([P, 4, P], F16, tag="on")
            for j in range(4):
                nc.vector.tensor_scalar_mul(
                    onx[:, j, :], uds[nch][:, j, :],
                    rinv[:, nch * 4 + j : nch * 4 + j + 1],
                )
            for j in range(4):
                # one transpose per psum tile: f16 transpose output cannot
                # share a psum accumulation group (HW faults)
                pst = pb.tile([P, P], F16, tag="pb", name="pst")
                nc.tensor.transpose(pst, onx[:, j, :], ident)
                nc.vector.tensor_copy(
                    outnT[:, u, nch * CH + j * P : nch * CH + (j + 1) * P], pst
                )

    if DEBUG:
        nc.gpsimd.dma_start(t["dbg_onT"], outnT[:, 0, :])
        nc.gpsimd.dma_start(t["dbg_onT1"], outnT[:, 1, :])

    # ---- phase 3: y[q, d] = sum_u outnT_u^T @ wo_u ----
    # one [128, 1024] row-tile per q-tile through the freed pa slots (2
    # banks each); full 4KB-contiguous DMA lines out.
    for qt in ([] if ABL == 'p1o' else range(MT)):
        qsl = slice(qt * P, (qt + 1) * P)
        yp = pa.tile([P, 2, 512], F32, tag="pa", name="y")
        for dc in range(2):
            dsl = slice(dc * 512, (dc + 1) * 512)
            for u in range(PAIRS):
                nc.tensor.matmul(
                    yp[:, dc, :],
                    lhsT=outnT[:, u, qsl],
                    rhs=wo_sb[:, u, dsl],
                    start=(u == 0),
                    stop=(u == PAIRS - 1),
                )
        ysb = ypool.tile([P, 2, 512], F32, tag="ysb")
        # split the eviction across both engines so the pa slot frees fast
        nc.vector.tensor_copy(ysb[:, 0, :], yp[:, 0, :])
        nc.scalar.activation(ysb[:, 1, :], yp[:, 1, :], Act.Identity)
        nc.sync.dma_start(
            t["ypart"][qsl, :].rearrange("p (a b) -> p a b", a=2), ysb
        )


def build_program(iters=1):
    nc = bacc.Bacc("TRN2", target_bir_lowering=False, debug=False)
    t = {
        "xt": nc.dram_tensor("xt", [DIM, N], F16, kind="ExternalInput")[:],
        "wq": nc.dram_tensor("wq", [DIM, PAIRS * D2], F16, kind="ExternalInput")[:],
        "wk": nc.dram_tensor("wk", [DIM, PAIRS * D2], F16, kind="ExternalInput")[:],
        "wv": nc.dram_tensor("wv", [DIM, PAIRS * D2], F16, kind="ExternalInput")[:],
        "wo": nc.dram_tensor("wo", [PAIRS * D2, DIM], F16, kind="ExternalInput")[:],
        "params": nc.dram_tensor("params", [P, 8], F32, kind="ExternalInput")[:],
        "bvr": nc.dram_tensor("bvr", [1, PAIRS * D2], F16, kind="ExternalInput")[:],
        "ypart": nc.dram_tensor("ypart", [N, DIM], F32, kind="ExternalOutput")[:],
    }
    if DEBUG:
        for nm, shp in [
            ("dbg_qt", [P, N]), ("dbg_kt", [P, N]), ("dbg_v", [P, MT * P]),
            ("dbg_rinv", [P, 16]),
            ("dbg_onT", [P, N]), ("dbg_onT1", [P, N]),
        ]:
            t[nm] = nc.dram_tensor(nm, shp, F32, kind="ExternalOutput")[:]
    with tile.TileContext(nc) as tc:
        _emit(tc, t, iters=iters)
    nc.compile()
    return nc


_NC_CACHE = {}


def _get_nc(iters=1):
    if iters not in _NC_CACHE:
        _NC_CACHE[iters] = build_program(iters)
    return _NC_CACHE[iters]


def make_core_inputs(x, Wq, bq, Wk, bk, Wv, bv, Wo, bo, g, lq1, lk1, lq2, lk2):
    """Host-side shard prep.  Returns (in_maps, lam) for the 8 cores."""
    x = np.asarray(x, np.float32)
    scaling = np.float32(HEAD_DIM**-0.5)
    lam1 = np.exp(np.sum(np.asarray(lq1, np.float32) * np.asarray(lk1, np.float32)))
    lam2 = np.exp(np.sum(np.asarray(lq2, np.float32) * np.asarray(lk2, np.float32)))
    lam = np.float32(lam1 - lam2 + LAMBDA_INIT)

    xt = np.ascontiguousarray(np.transpose(x, (0, 2, 1)))  # (B, DIM, N)
    Wq_s = np.asarray(Wq, np.float32) * scaling
    bq_s = np.asarray(bq, np.float32) * scaling
    geff = (np.asarray(g, np.float32) * np.float32(1.0 - LAMBDA_INIT)).reshape(P)

    in_maps = []
    for c in range(CORES):
        b = c // 4
        grp = c % 4
        cols = slice(grp * PAIRS * D2, (grp + 1) * PAIRS * D2)
        params = np.zeros((P, 8), np.float32)
        params[:, 0:2] = bq_s[cols].reshape(PAIRS, P).T
        params[:, 2:4] = np.asarray(bk, np.float32)[cols].reshape(PAIRS, P).T
        params[:, 7] = -lam
        # Wo rows for this core, pre-scaled by g*(1-lam_init) per vd
        wo_eff = np.asarray(Wo, np.float32)[cols, :] * np.tile(geff, PAIRS)[:, None]
        in_maps.append(
            {
                "xt": np.ascontiguousarray(xt[b]).astype(np.float16),
                "wq": np.ascontiguousarray(Wq_s[:, cols]).astype(np.float16),
                "wk": np.ascontiguousarray(
                    np.asarray(Wk, np.float32)[:, cols]
                ).astype(np.float16),
                "wv": np.ascontiguousarray(
                    np.asarray(Wv, np.float32)[:, cols]
                ).astype(np.float16),
                "wo": np.ascontiguousarray(wo_eff).astype(np.float16),
                "params": params,
                "bvr": np.asarray(bv, np.float32)[cols]
                .reshape(1, PAIRS * D2)
                .astype(np.float16),
            }
        )
    return in_maps, lam


def gather_output(results, bo):
    """Sum per-core y partials per batch, add bo."""
    bo = np.asarray(bo, np.float32)
    out = np.empty((B, N, DIM), np.float32)
    for b in range(B):
        acc = np.zeros((N, DIM), np.float32)
        for c in range(b * 4, b * 4 + 4):
            acc += results[c]["ypart"]
        out[b] = acc + bo
    return out


_IN_CACHE = {}


def kernel(**inputs):
    from concourse.bass_utils import run_bass_kernel_spmd

    key = id(inputs.get("x"))
    if key in _IN_CACHE:
        in_maps = _IN_CACHE[key]
    else:
        in_maps, _ = make_core_inputs(**inputs)
        _IN_CACHE.clear()
        _IN_CACHE[key] = in_maps
    iters = int(os.environ.get("KERNEL_ITERS", "1"))
    nc = _get_nc(iters)
    trace = bool(int(os.environ.get("KERNEL_TRACE", "0")))
    res = run_bass_kernel_spmd(
        nc, in_maps, core_ids=list(range(CORES)), trace=trace
    )
    if trace and res.exec_time_ns is not None:
        print(f"HW exec time: {res.exec_time_ns} ns")
        kernel.last_exec_time_ns = res.exec_time_ns
        kernel.last_trace = res.instructions_and_trace
    return gather_output(res.results, inputs["bo"])


# ---------------- dev helpers (not used by the grading harness) ----------------


def _numpy_core_partial(im):
    """Reference computation of one core's ypart from its sharded inputs."""
    xt = im["xt"].astype(np.float64)  # [DIM, N]
    x = xt.T
    pr = im["params"]
    lam = -float(pr[0, 7])
    ypart = np.zeros((N, DIM))
    for u in range(PAIRS):
        usl = slice(u * D2, (u + 1) * D2)
        q = x @ im["wq"][:, usl].astype(np.float64) + pr[:, u]  # [N, 128]
        k = x @ im["wk"][:, usl].astype(np.float64) + pr[:, 2 + u]
        v = x @ im["wv"][:, usl].astype(np.float64) + im["bvr"][0, usl].astype(
            np.float64
        )
        s1 = q[:, :64] @ k[:, :64].T
        s2 = q[:, 64:] @ k[:, 64:].T
        p1 = np.exp(s1)
        p1 /= p1.sum(-1, keepdims=True)
        p2 = np.exp(s2)
        p2 /= p2.sum(-1, keepdims=True)
        diff = p1 - lam * p2
        o = diff @ v  # [N, 128]
        rms = 1.0 / np.sqrt((o * o).mean(-1, keepdims=True) + EPS)
        o = o * rms
        ypart += o @ im["wo"][usl, :].astype(np.float64)
    return ypart


if __name__ == "__main__":
    mode = sys.argv[1] if len(sys.argv) > 1 else "sim"
    sys.path.insert(0, "/root/problem")
    import reference

    inputs = {k: np.asarray(v) for k, v in reference.setup_inputs().items()}
    in_maps, lam = make_core_inputs(**inputs)
    print("lam =", lam)
    nc = _get_nc()
    if mode == "sim":
        from concourse.bass_interp import CoreSim

        sim = CoreSim(nc)
        for k, v in in_maps[0].items():
            sim.tensor(k)[:] = v
        sim.simulate()
        got = np.array(sim.tensor("ypart"))
        want = _numpy_core_partial(in_maps[0])
        err = np.abs(got - want)
        scale = np.abs(want).max()
        print("absmax err:", err.max(), "rel:", err.max() / scale, "scale:", scale)
        try:
            print("sim predicted time:", sim.time, "ns")
        except Exception as e:
            print("no sim time:", e)



# revision 46
# speedup vs baseline: 1.2381x; 1.2381x over previous
"""DiffAttention TRN2 Bass kernel, v3.

Problem: nn_DiffAttention_15977278341927
  B=2, N=2048, DIM=1024, 16 heads of dim 64 -> 8 effective head-pairs.
  out = ((softmax(Q1K1^T) - lam*softmax(Q2K2^T)) @ V) -> headwise RMSNorm
        -> *(1-lam_init) -> concat heads -> @ Wo + bo

Sharding (8 cores): batch (2) x head-pair groups (4 groups of 2 pairs).
Core c: batch c//4, pairs {2*(c%4), 2*(c%4)+1}.  QKV col-sharded, Wo
row-sharded (pre-scaled by g*(1-lam_init) on host), partials summed on host.

v3 changes over v2:
  - Softmax denominators folded into PV: V_sb carries a ones column
    (129-wide moving operand), so den = Up[:, k, 128].  Kills the 1024
    1-col matmuls and their stationary reloads that serialized PV.
  - V^T produced once for both pairs (256-col moving) instead of per
    pair (128-col).
  - All softmax exp on the Activation engine (the DVE poly-exp cost
    1836ns/tile vs 1125ns on Act; PE is the bottleneck, not Act).
  - xt DMA split per 512-token chunk so phase 1 starts ~6us in.
  - f16 ypart output (host accumulates partials in f32).
"""

import os
import sys

sys.path.insert(0, "/opt/trn_rl_repo")

import numpy as np

import concourse.bacc as bacc
import concourse.bass as bass
import concourse.mybir as mybir
import concourse.tile as tile
from concourse.masks import make_identity

B, N, DIM = 2, 2048, 1024
EFF = 8
HEAD_DIM = 64
D2 = 2 * HEAD_DIM  # 128: one head-pair's q/k cols and one v head's dims
LAMBDA_INIT = 0.8
EPS = 1e-5

P = 128
CH = 512  # S^T tile q-chunk
NCH = N // CH  # 4
MT = N // P  # 16 key tiles
KT_D = DIM // P  # 8 contraction tiles over DIM
PAIRS = 2
CORES = 8
VW = 132  # V_sb row stride (128 vd + ones col at 128 + pad)
US = 170  # Up accumulator slot stride in f32 (129 used, 3*170*4 <= 2048)

F32 = mybir.dt.float32
F16 = mybir.dt.float16
DEBUG = bool(int(os.environ.get("KERNEL_DEBUG", "0")))


def _emit(tc, t, iters=1):
    from contextlib import ExitStack
    import contextlib

    nc = tc.nc

    ctx = ExitStack()
    with ctx:
        singles = ctx.enter_context(tc.tile_pool(name="singles", bufs=1))
        big = ctx.enter_context(tc.tile_pool(name="big", bufs=2))
        expp = ctx.enter_context(tc.tile_pool(name="expp", bufs=6))
        udp = ctx.enter_context(tc.tile_pool(name="udp", bufs=2))
        onp = ctx.enter_context(tc.tile_pool(name="onp", bufs=2))
        sm = ctx.enter_context(tc.tile_pool(name="sm", bufs=4))
        ypool = ctx.enter_context(tc.tile_pool(name="ypool", bufs=4))
        pa = ctx.enter_context(tc.tile_pool(name="pa", bufs=2, space="PSUM"))
        pu = ctx.enter_context(tc.tile_pool(name="pu", bufs=1, space="PSUM"))
        pb = ctx.enter_context(tc.tile_pool(name="pb", bufs=1, space="PSUM"))

        # ---- loads / constants ----
        # All DRAM operands are pre-arranged on the host so every DMA reads
        # fully contiguous per-partition rows.  xt chunks go on the sync
        # HWDGE queue, weights on the activation HWDGE queue, scalars on
        # gpsimd: the first matmul waits only on wq + xt chunk 0.
        # transfers are FIFO per ring (~9us/MB incl. completion latency), so
        # spread the startup-critical loads across all three queues in
        # first-use order: sync {xt0, wv, xt2}, act {wq, xt1, wk, xt3},
        # gpsimd {params, bvr, wo}.
        xt_sb = []
        w_sb = {}

        def _load_xt(nch, eng):
            xc = singles.tile([P, KT_D, CH], F16, name=f"xt{nch}")
            eng.dma_start(
                xc,
                t["xt"][nch * P : (nch + 1) * P, :].rearrange(
                    "p (k c) -> p k c", k=KT_D
                ),
            )
            xt_sb.append(xc)

        def _load_w(w, eng):
            w_sb[w] = singles.tile([P, KT_D, PAIRS * D2], F16, name=f"w_{w}")
            eng.dma_start(w_sb[w], t[w].rearrange("p (k c) -> p k c", k=KT_D))

        _load_xt(0, nc.sync)
        _load_w("wq", nc.scalar)
        _load_xt(1, nc.scalar)
        _load_w("wk", nc.scalar)
        _load_w("wv", nc.sync)
        _load_xt(2, nc.sync)
        _load_xt(3, nc.scalar)
        wo_sb = singles.tile([P, PAIRS, DIM], F16)
        nc.gpsimd.dma_start(wo_sb, t["wo"].rearrange("p (u c) -> p u c", u=PAIRS))
        params_sb = singles.tile([P, 8], F32)
        nc.gpsimd.dma_start(params_sb, t["params"])
        bvr_sb = singles.tile([1, PAIRS * D2], F16)
        nc.gpsimd.dma_start(bvr_sb, t["bvr"])
        # bv broadcast across partitions once at setup; added during the
        # V^T psum eviction (varies along the free dim there)
        bvb_sb = singles.tile([P, PAIRS, D2], F16)
        nc.gpsimd.partition_broadcast(
            bvb_sb.rearrange("p a b -> p (a b)"), bvr_sb, channels=P
        )
        ident = singles.tile([P, P], F16)
        make_identity(nc, ident)
        # V^T for both pairs, with a ones column at 128 of each VW slot
        # (the PV moving operand is 129 wide; col 128 yields the softmax
        # denominator in Up[:, k, 128]).
        V_sb = singles.tile([P, MT, PAIRS, VW], F16)
        nc.vector.memset(V_sb[:, :, :, D2 : D2 + 1], 1.0)
        # transposed normalized outputs, [vd, pair, n]
        outnT = singles.tile([P, PAIRS, N], F16)

        env = dict(locals())
        loop_cm = (
            tc.For_i(0, iters, 1, hint_engines=(mybir.EngineType.PE,))
            if iters > 1
            else contextlib.nullcontext()
        )
        with loop_cm:
            _emit_body(tc, t, env)


def _emit_body(tc, t, env):
    nc = tc.nc
    Act = mybir.ActivationFunctionType
    Alu = mybir.AluOpType
    big = env["big"]; expp = env["expp"]; udp = env["udp"]; onp = env["onp"]
    sm = env["sm"]; ypool = env["ypool"]
    pa = env["pa"]; pu = env["pu"]; pb = env["pb"]
    xt_sb = env["xt_sb"]; w_sb = env["w_sb"]; wo_sb = env["wo_sb"]
    params_sb = env["params_sb"]; bvb_sb = env["bvb_sb"]
    ident = env["ident"]; V_sb = env["V_sb"]; outnT = env["outnT"]

    # ---- phase 1: QT,KT [d-pair, n] per pair; V^T [token, pair, vd] ----
    # V^T matmul groups (single-buffered pb psum) are interleaved between
    # the Q/K accumulation groups so their DVE evictions hide under PE work.
    QTs, KTs = [], []
    for u in range(PAIRS):
        QT = big.tile([P, N], F16, tag=f"QT{u}", name=f"QT{u}")
        KTt = big.tile([P, N], F16, tag=f"KT{u}", name=f"KT{u}")
        QTs.append(QT)
        KTs.append(KTt)

    def _emit_vt(nch, i):
        tt = nch * (CH // P) + i
        xc = xt_sb[nch]
        vt = pb.tile([P, PAIRS, D2], F32, tag="pb", name="vt")
        for kt in range(KT_D):
            nc.tensor.matmul(
                vt.rearrange("p a b -> p (a b)"),
                lhsT=xc[:, kt, i * P : (i + 1) * P],
                rhs=w_sb["wv"][:, kt, :],
                start=(kt == 0),
                stop=(kt == KT_D - 1),
            )
        nc.vector.tensor_tensor(V_sb[:, tt, :, 0:D2], vt, bvb_sb, Alu.add)

    for nch in range(NCH):
        sl = slice(nch * CH, (nch + 1) * CH)
        xc = xt_sb[nch]
        for u in range(PAIRS):
            usl = slice(u * D2, (u + 1) * D2)
            ps = pa.tile([P, 2, CH], F32, tag="pa")
            for kt in range(KT_D):
                nc.tensor.matmul(
                    ps[:, 0, :],
                    lhsT=w_sb["wq"][:, kt, usl],
                    rhs=xc[:, kt, :],
                    start=(kt == 0),
                    stop=(kt == KT_D - 1),
                )
            _emit_vt(nch, 2 * u)
            for kt in range(KT_D):
                nc.tensor.matmul(
                    ps[:, 1, :],
                    lhsT=w_sb["wk"][:, kt, usl],
                    rhs=xc[:, kt, :],
                    start=(kt == 0),
                    stop=(kt == KT_D - 1),
                )
            _emit_vt(nch, 2 * u + 1)
            nc.vector.tensor_scalar_add(
                QTs[u][:, sl], ps[:, 0, :], params_sb[:, u : u + 1]
            )
            nc.vector.tensor_scalar_add(
                KTs[u][:, sl], ps[:, 1, :], params_sb[:, 2 + u : 3 + u]
            )

    if DEBUG:
        nc.gpsimd.dma_start(t["dbg_qt"], QTs[0])
        nc.gpsimd.dma_start(t["dbg_kt"], KTs[0])
        nc.gpsimd.dma_start(
            t["dbg_v"], V_sb[:, :, 0, 0:D2].rearrange("p a b -> p (a b)")
        )

    # Up accumulator layout: k = 4*h + j -> tile k//3, slot k%3.
    # Each slot is 129 f32 (128 vd + den), strided US to stay in-bank.
    def _up_slot(ups, k):
        return ups[k // 3][:, k % 3, 0:129]

    # ---- phase 2 (both pairs): S^T -> exp -> U[q,vd]+den via ones col ----
    # Each pair's onx-scale + transpose work is DEFERRED: emitted one chunk
    # at a time inside the next pair's chunk loop (pair 1: inside phase 3),
    # so the PE transposes interleave with S/PV matmuls instead of idling
    # behind the rinv(Act)/onx(DVE) chain.
    deferred = []

    def _emit_out(u, nch, ud, rinv):
        def emit():
            onx = onp.tile([P, 4, P], F16, tag="on", name="on")
            for j in range(4):
                nc.vector.tensor_scalar_mul(
                    onx[:, j, :], ud[:, j, :],
                    rinv[:, nch * 4 + j : nch * 4 + j + 1],
                )
            for j in range(4):
                # one transpose per psum tile: f16 transpose output cannot
                # share a psum accumulation group (HW faults).  (XBAR DMA
                # transpose measured 20us slower end-to-end — reverted.)
                pst = pb.tile([P, P], F16, tag="pb", name="pst")
                nc.tensor.transpose(pst, onx[:, j, :], ident)
                nc.vector.tensor_copy(
                    outnT[:, u, nch * CH + j * P : nch * CH + (j + 1) * P],
                    pst,
                )
        return emit

    for u in range(PAIRS):
        QT, KTt = QTs[u], KTs[u]
        uds = []
        msq = sm.tile([P, MT], F32, tag="msq", bufs=2)
        epsq_all = sm.tile([P, MT], F32, tag="epsq_all", bufs=2)
        for nch in range(NCH):
            # PE queue is strict FIFO: only emit deferred transposes where
            # their rinv chain (act) is guaranteed drained — pair-0's items
            # pop inside pair-1's later chunks, pair-1's inside phase 3.
            if u == 1 and nch >= 1:
                deferred.pop(0)()
            sl = slice(nch * CH, (nch + 1) * CH)
            ups = [
                pu.tile([P, 3, US], F32, tag="upA", name="upA"),
                pu.tile([P, 3, US], F32, tag="upB", name="upB"),
                pu.tile([P, 3, US], F32, tag="upC", name="upC"),
            ]

            exq = {}

            def _issue_exp(mt, sp):
                ex = expp.tile([P, 2, CH], F16, tag="ex", name="ex")
                # All exp on the Activation engine, one full [P,2,512] tile
                # per instr.  Measured dead ends: DVE custom-op exp (2.4us/
                # tile, FIFO stalls), strided act split (819ns for 768 elems
                # — worse than 1125ns for 1024).
                nc.scalar.activation(ex, sp, Act.Exp)
                exq[mt] = ex

            def _issue_pv(mt):
                ex = exq.pop(mt)
                for h in range(2):
                    for j in range(CH // P):
                        lh = ex[:, h, j * P : (j + 1) * P]
                        k = 4 * h + j
                        # one start/stop per psum tile (zero region):
                        # first/last k emitted into tile k//3 is k%3==0 /
                        # k%3==2 (tile 2 ends at k=7, k%3==1).
                        nc.tensor.matmul(
                            _up_slot(ups, k),
                            lhsT=lh,
                            rhs=V_sb[:, mt, u, 0 : D2 + 1],
                            start=(mt == 0 and k % 3 == 0),
                            stop=(mt == MT - 1 and (k % 3 == 2 or k == 7)),
                        )

            for mt in range(MT):
                msl = slice(mt * P, (mt + 1) * P)
                sp = pa.tile([P, 2, CH], F32, tag="pa")
                nc.tensor.matmul(
                    sp[:, 0, :], lhsT=KTt[0:64, msl], rhs=QT[0:64, sl]
                )
                nc.tensor.matmul(
                    sp[:, 1, :], lhsT=KTt[64:128, msl], rhs=QT[64:128, sl]
                )
                _issue_exp(mt, sp)
                if mt >= 2:
                    _issue_pv(mt - 2)
            _issue_pv(MT - 2)
            _issue_pv(MT - 1)

            # ---- post-processing: all per-partition [128, 4] ops ----
            # order: Up readers (ud) first so the single-buffered pu frees
            # for the next chunk's accumulation as early as possible.
            # DVE may read only ONE non-scalar input from PSUM per op:
            # stage s1 and U1 to SBUF first.
            # s1 = den of k0..3 = A0,A1,A2,B0 @ col 128; s2 = B1,B2,C0,C1.
            s1c = sm.tile([P, 4], F32, tag="s1c")
            nc.vector.tensor_copy(s1c[:, 0:3], ups[0][:, :, 128])
            nc.vector.tensor_copy(s1c[:, 3:4], ups[1][:, 0:1, 128])
            r2 = sm.tile([P, 4], F32, tag="r2")
            nc.vector.reciprocal(r2[:, 0:2], ups[1][:, 1:3, 128])
            nc.vector.reciprocal(r2[:, 2:4], ups[2][:, 0:2, 128])
            # f = -lam * s1 / s2   (params[:,7] = -lam)
            f = sm.tile([P, 4], F32, tag="f")
            nc.vector.scalar_tensor_tensor(
                out=f, in0=s1c, scalar=params_sb[:, 7:8],
                in1=r2, op0=Alu.mult, op1=Alu.mult,
            )
            u1c = udp.tile([P, 4, P], F32, tag="u1c")
            nc.vector.tensor_copy(u1c[:, 0:3, :], ups[0][:, :, 0:128])
            nc.vector.tensor_copy(u1c[:, 3, :], ups[1][:, 0, 0:128])
            ud = udp.tile([P, 4, P], F16, tag="ud", bufs=5)
            for j in range(4):
                # ud = U1 + f*U2  (f negated)
                nc.vector.scalar_tensor_tensor(
                    out=ud[:, j, :], in0=_up_slot(ups, 4 + j)[:, 0:128],
                    scalar=f[:, j : j + 1], in1=u1c[:, j, :],
                    op0=Alu.mult, op1=Alu.add,
                )
            # epsq = (P*eps) * s1^2  (added to ssq per pair, before Ln)
            nc.vector.scalar_tensor_tensor(
                out=epsq_all[:, nch * 4 : nch * 4 + 4], in0=s1c,
                scalar=float(P) * EPS,
                in1=s1c, op0=Alu.mult, op1=Alu.mult,
            )
            # ssq: square (f16 fast) + free-axis reduce; tensor_tensor_reduce
            # faults TRN2 here, so use the two-op form
            sqscr = udp.tile([P, 4, P], F32, tag="sq")
            nc.vector.tensor_mul(sqscr, ud, ud)
            uds.append(ud)
            for j in range(4):
                nc.vector.reduce_sum(
                    out=msq[:, nch * 4 + j : nch * 4 + j + 1],
                    in_=sqscr[:, j, :],
                    axis=mybir.AxisListType.X,
                )

        # rinv = rsqrt((ssq + P*eps*s1^2) / P) = exp(-0.5 * ln(msqe/P))
        # batched per pair ([P,16]): per-chunk [P,4] Ln/Exp measured 790ns
        # each (overhead-dominated) = +19us of act — batched is 2 ops/pair
        nc.vector.tensor_add(msq, msq, epsq_all)
        lnm = sm.tile([P, MT], F32, tag="lnm", bufs=2)
        nc.scalar.activation(lnm, msq, Act.Ln, scale=1.0 / P)
        rinv = sm.tile([P, MT], F32, tag="rinv", bufs=2)
        nc.scalar.activation(rinv, lnm, Act.Exp, scale=-0.5)
        for nch in range(NCH):
            deferred.append(_emit_out(u, nch, uds[nch], rinv))

    # ---- phase 3: y[q, d] = sum_u outnT_u^T @ wo_u ----
    # one [128, 1024] row-tile per q-tile through the freed pa slots (2
    # banks each); full contiguous DMA lines out.  Pair 1's deferred
    # onx/transpose for chunk c is emitted just before chunk c's q-tiles.
    for qt in range(MT):
        # remaining deferred outputs (pair-0 chunk 3 + all of pair-1):
        # p1 chunk c's transposes must land before qt==4c; qt 0 takes two
        if qt == 0:
            deferred.pop(0)()  # p0c3
            deferred.pop(0)()  # p1c0
        elif qt in (4, 8, 12):
            deferred.pop(0)()  # p1c(qt//4)
        qsl = slice(qt * P, (qt + 1) * P)
        yp = pa.tile([P, 2, 512], F32, tag="pa", name="y")
        for dc in range(2):
            dsl = slice(dc * 512, (dc + 1) * 512)
            for u in range(PAIRS):
                nc.tensor.matmul(
                    yp[:, dc, :],
                    lhsT=outnT[:, u, qsl],
                    rhs=wo_sb[:, u, dsl],
                    start=(u == 0),
                    stop=(u == PAIRS - 1),
                )
        ysb = ypool.tile([P, 2, 512], F16, tag="ysb")
        # split the eviction across both engines so the pa slot frees fast
        nc.vector.tensor_copy(ysb[:, 0, :], yp[:, 0, :])
        nc.scalar.activation(ysb[:, 1, :], yp[:, 1, :], Act.Identity)
        # gpsimd SWDGE enqueue is async (~430ns engine time) vs the sync
        # HWDGE dma_start which blocks the queue for the HBM-write latency
        nc.gpsimd.dma_start(
            t["ypart"][qsl, :].rearrange("p (a b) -> p a b", a=2), ysb
        )

    if DEBUG:
        nc.gpsimd.dma_start(t["dbg_onT"], outnT[:, 0, :])
        nc.gpsimd.dma_start(t["dbg_onT1"], outnT[:, 1, :])


def build_program(iters=1):
    nc = bacc.Bacc("TRN2", target_bir_lowering=False, debug=False)
    t = {
        # host pre-arranged: xt[n*128+p, k*512+c]; w*[p, k*256+c]; wo[p, u*1024+c]
        "xt": nc.dram_tensor("xt", [NCH * P, KT_D * CH], F16, kind="ExternalInput")[:],
        "wq": nc.dram_tensor("wq", [P, KT_D * PAIRS * D2], F16, kind="ExternalInput")[:],
        "wk": nc.dram_tensor("wk", [P, KT_D * PAIRS * D2], F16, kind="ExternalInput")[:],
        "wv": nc.dram_tensor("wv", [P, KT_D * PAIRS * D2], F16, kind="ExternalInput")[:],
        "wo": nc.dram_tensor("wo", [P, PAIRS * DIM], F16, kind="ExternalInput")[:],
        "params": nc.dram_tensor("params", [P, 8], F32, kind="ExternalInput")[:],
        "bvr": nc.dram_tensor("bvr", [1, PAIRS * D2], F16, kind="ExternalInput")[:],
        "ypart": nc.dram_tensor("ypart", [N, DIM], F16, kind="ExternalOutput")[:],
    }
    if DEBUG:
        for nm, shp in [
            ("dbg_qt", [P, N]), ("dbg_kt", [P, N]), ("dbg_v", [P, MT * P]),
            ("dbg_rinv", [P, 16]),
            ("dbg_onT", [P, N]), ("dbg_onT1", [P, N]),
        ]:
            t[nm] = nc.dram_tensor(nm, shp, F32, kind="ExternalOutput")[:]
    with tile.TileContext(nc) as tc:
        _emit(tc, t, iters=iters)
    nc.compile()
    return nc


_NC_CACHE = {}


def _get_nc(iters=1):
    if iters not in _NC_CACHE:
        _NC_CACHE[iters] = build_program(iters)
    return _NC_CACHE[iters]


def make_core_inputs(x, Wq, bq, Wk, bk, Wv, bv, Wo, bo, g, lq1, lk1, lq2, lk2):
    """Host-side shard prep.  Returns (in_maps, lam) for the 8 cores."""
    x = np.asarray(x, np.float32)
    scaling = np.float32(HEAD_DIM**-0.5)
    lam1 = np.exp(np.sum(np.asarray(lq1, np.float32) * np.asarray(lk1, np.float32)))
    lam2 = np.exp(np.sum(np.asarray(lq2, np.float32) * np.asarray(lk2, np.float32)))
    lam = np.float32(lam1 - lam2 + LAMBDA_INIT)

    xt = np.ascontiguousarray(np.transpose(x, (0, 2, 1)))  # (B, DIM, N)
    Wq_s = np.asarray(Wq, np.float32) * scaling
    bq_s = np.asarray(bq, np.float32) * scaling
    geff = (np.asarray(g, np.float32) * np.float32(1.0 - LAMBDA_INIT)).reshape(P)

    in_maps = []
    for c in range(CORES):
        b = c // 4
        grp = c % 4
        cols = slice(grp * PAIRS * D2, (grp + 1) * PAIRS * D2)
        params = np.zeros((P, 8), np.float32)
        params[:, 0:2] = bq_s[cols].reshape(PAIRS, P).T
        params[:, 2:4] = np.asarray(bk, np.float32)[cols].reshape(PAIRS, P).T
        params[:, 7] = -lam
        # Wo rows for this core, pre-scaled by g*(1-lam_init) per vd
        wo_eff = np.asarray(Wo, np.float32)[cols, :] * np.tile(geff, PAIRS)[:, None]

        def _wlay(W):  # [DIM, C] -> [P, KT_D*C]: w[p, k*C+c] = W[k*128+p, c]
            C = W.shape[1]
            return np.ascontiguousarray(
                W.reshape(KT_D, P, C).transpose(1, 0, 2).reshape(P, KT_D * C)
            )

        # xt chunk-contiguous: xtc[n*128+p, k*512+c] = xt[k*128+p, n*512+c]
        xtc = (
            xt[b]
            .reshape(KT_D, P, NCH, CH)
            .transpose(2, 1, 0, 3)
            .reshape(NCH * P, KT_D * CH)
        )
        in_maps.append(
            {
                "xt": np.ascontiguousarray(xtc).astype(np.float16),
                "wq": _wlay(Wq_s[:, cols]).astype(np.float16),
                "wk": _wlay(np.asarray(Wk, np.float32)[:, cols]).astype(np.float16),
                "wv": _wlay(np.asarray(Wv, np.float32)[:, cols]).astype(np.float16),
                "wo": np.ascontiguousarray(
                    wo_eff.reshape(PAIRS, P, DIM)
                    .transpose(1, 0, 2)
                    .reshape(P, PAIRS * DIM)
                ).astype(np.float16),
                "params": params,
                "bvr": np.asarray(bv, np.float32)[cols]
                .reshape(1, PAIRS * D2)
                .astype(np.float16),
            }
        )
    return in_maps, lam


def gather_output(results, bo):
    """Sum per-core y partials per batch, add bo."""
    bo = np.asarray(bo, np.float32)
    out = np.empty((B, N, DIM), np.float32)
    for b in range(B):
        acc = np.zeros((N, DIM), np.float32)
        for c in range(b * 4, b * 4 + 4):
            acc += results[c]["ypart"].astype(np.float32)
        out[b] = acc + bo
    return out


_IN_CACHE = {}


def kernel(**inputs):
    from concourse.bass_utils import run_bass_kernel_spmd

    key = id(inputs.get("x"))
    if key in _IN_CACHE:
        in_maps = _IN_CACHE[key]
    else:
        in_maps, _ = make_core_inputs(**inputs)
        _IN_CACHE.clear()
        _IN_CACHE[key] = in_maps
    iters = int(os.environ.get("KERNEL_ITERS", "1"))
    nc = _get_nc(iters)
    trace = bool(int(os.environ.get("KERNEL_TRACE", "0")))
    res = run_bass_kernel_spmd(
        nc, in_maps, core_ids=list(range(CORES)), trace=trace
    )
    if trace and res.exec_time_ns is not None:
        print(f"HW exec time: {res.exec_time_ns} ns")
        kernel.last_exec_time_ns = res.exec_time_ns
        kernel.last_trace = res.instructions_and_trace
    return gather_output(res.results, inputs["bo"])


# ---------------- dev helpers (not used by the grading harness) ----------------


def _numpy_core_partial(im):
    """Reference computation of one core's ypart from its sharded inputs."""
    # undo the host DMA layouts
    xt = (
        im["xt"]
        .reshape(NCH, P, KT_D, CH)
        .transpose(2, 1, 0, 3)
        .reshape(DIM, N)
        .astype(np.float64)
    )
    wq = im["wq"].reshape(P, KT_D, -1).transpose(1, 0, 2).reshape(DIM, -1)
    wk = im["wk"].reshape(P, KT_D, -1).transpose(1, 0, 2).reshape(DIM, -1)
    wv = im["wv"].reshape(P, KT_D, -1).transpose(1, 0, 2).reshape(DIM, -1)
    wo = im["wo"].reshape(P, PAIRS, DIM).transpose(1, 0, 2).reshape(PAIRS * D2, DIM)
    x = xt.T
    pr = im["params"]
    lam = -float(pr[0, 7])
    ypart = np.zeros((N, DIM))
    for u in range(PAIRS):
        usl = slice(u * D2, (u + 1) * D2)
        q = x @ wq[:, usl].astype(np.float64) + pr[:, u]  # [N, 128]
        k = x @ wk[:, usl].astype(np.float64) + pr[:, 2 + u]
        v = x @ wv[:, usl].astype(np.float64) + im["bvr"][0, usl].astype(
            np.float64
        )
        s1 = q[:, :64] @ k[:, :64].T
        s2 = q[:, 64:] @ k[:, 64:].T
        p1 = np.exp(s1)
        p1 /= p1.sum(-1, keepdims=True)
        p2 = np.exp(s2)
        p2 /= p2.sum(-1, keepdims=True)
        diff = p1 - lam * p2
        o = diff @ v  # [N, 128]
        rms = 1.0 / np.sqrt((o * o).mean(-1, keepdims=True) + EPS)
        o = o * rms
        ypart += o @ wo[usl, :].astype(np.float64)
    return ypart


if __name__ == "__main__":
    mode = sys.argv[1] if len(sys.argv) > 1 else "sim"
    sys.path.insert(0, "/root/problem")
    import reference

    inputs = {k: np.asarray(v) for k, v in reference.setup_inputs().items()}
    in_maps, lam = make_core_inputs(**inputs)
    print("lam =", lam)
    nc = _get_nc()
    if mode == "sim":
        from concourse.bass_interp import CoreSim

        sim = CoreSim(nc)
        for k, v in in_maps[0].items():
            sim.tensor(k)[:] = v
        sim.simulate()
        got = np.array(sim.tensor("ypart")).astype(np.float64)
        want = _numpy_core_partial(in_maps[0])
        err = np.abs(got - want)
        scale = np.abs(want).max()
        print("absmax err:", err.max(), "rel:", err.max() / scale, "scale:", scale)
        try:
            print("sim predicted time:", sim.time, "ns")
        except Exception as e:
            print("no sim time:", e)


# revision 52
# speedup vs baseline: 1.2541x; 1.0129x over previous
"""DiffAttention TRN2 Bass kernel, v3.

Problem: nn_DiffAttention_15977278341927
  B=2, N=2048, DIM=1024, 16 heads of dim 64 -> 8 effective head-pairs.
  out = ((softmax(Q1K1^T) - lam*softmax(Q2K2^T)) @ V) -> headwise RMSNorm
        -> *(1-lam_init) -> concat heads -> @ Wo + bo

Sharding (8 cores): batch (2) x head-pair groups (4 groups of 2 pairs).
Core c: batch c//4, pairs {2*(c%4), 2*(c%4)+1}.  QKV col-sharded, Wo
row-sharded (pre-scaled by g*(1-lam_init) on host), partials summed on host.

v3 changes over v2:
  - Softmax denominators folded into PV: V_sb carries a ones column
    (129-wide moving operand), so den = Up[:, k, 128].  Kills the 1024
    1-col matmuls and their stationary reloads that serialized PV.
  - V^T produced once for both pairs (256-col moving) instead of per
    pair (128-col).
  - All softmax exp on the Activation engine (the DVE poly-exp cost
    1836ns/tile vs 1125ns on Act; PE is the bottleneck, not Act).
  - xt DMA split per 512-token chunk so phase 1 starts ~6us in.
  - f16 ypart output (host accumulates partials in f32).
"""

import os
import sys

sys.path.insert(0, "/opt/trn_rl_repo")

import numpy as np

import concourse.bacc as bacc
import concourse.bass as bass
import concourse.mybir as mybir
import concourse.tile as tile
from concourse.masks import make_identity

B, N, DIM = 2, 2048, 1024
EFF = 8
HEAD_DIM = 64
D2 = 2 * HEAD_DIM  # 128: one head-pair's q/k cols and one v head's dims
LAMBDA_INIT = 0.8
EPS = 1e-5

P = 128
CH = 512  # S^T tile q-chunk
NCH = N // CH  # 4
MT = N // P  # 16 key tiles
KT_D = DIM // P  # 8 contraction tiles over DIM
PAIRS = 2
CORES = 8
VW = 132  # V_sb row stride (128 vd + ones col at 128 + pad)
US = 170  # Up accumulator slot stride in f32 (129 used, 3*170*4 <= 2048)

F32 = mybir.dt.float32
F16 = mybir.dt.float16
DEBUG = bool(int(os.environ.get("KERNEL_DEBUG", "0")))


def _emit(tc, t, iters=1):
    from contextlib import ExitStack
    import contextlib

    nc = tc.nc

    ctx = ExitStack()
    with ctx:
        singles = ctx.enter_context(tc.tile_pool(name="singles", bufs=1))
        big = ctx.enter_context(tc.tile_pool(name="big", bufs=2))
        expp = ctx.enter_context(tc.tile_pool(name="expp", bufs=6))
        udp = ctx.enter_context(tc.tile_pool(name="udp", bufs=2))
        onp = ctx.enter_context(tc.tile_pool(name="onp", bufs=2))
        sm = ctx.enter_context(tc.tile_pool(name="sm", bufs=4))
        ypool = ctx.enter_context(tc.tile_pool(name="ypool", bufs=4))
        pa = ctx.enter_context(tc.tile_pool(name="pa", bufs=2, space="PSUM"))
        pu = ctx.enter_context(tc.tile_pool(name="pu", bufs=1, space="PSUM"))
        pb = ctx.enter_context(tc.tile_pool(name="pb", bufs=1, space="PSUM"))

        # ---- loads / constants ----
        # All DRAM operands are pre-arranged on the host so every DMA reads
        # fully contiguous per-partition rows.  xt chunks go on the sync
        # HWDGE queue, weights on the activation HWDGE queue, scalars on
        # gpsimd: the first matmul waits only on wq + xt chunk 0.
        # transfers are FIFO per ring (~9us/MB incl. completion latency), so
        # spread the startup-critical loads across all three rings so each
        # operand lands just before its first use (~9us/MB FIFO per ring):
        # sync {xt0, wk, xt3}, act {wq, xt1}, gpsimd {params, bvr, wv, wo,
        # xt2}.  Phase 1 emits Q,Q,V,K,K per chunk to push wk's deadline.
        xt_sb = []
        w_sb = {}

        def _load_xt(nch, eng):
            xc = singles.tile([P, KT_D, CH], F16, name=f"xt{nch}")
            eng.dma_start(
                xc,
                t["xt"][nch * P : (nch + 1) * P, :].rearrange(
                    "p (k c) -> p k c", k=KT_D
                ),
            )
            xt_sb.append(xc)

        def _load_w(w, eng):
            w_sb[w] = singles.tile([P, KT_D, PAIRS * D2], F16, name=f"w_{w}")
            eng.dma_start(w_sb[w], t[w].rearrange("p (k c) -> p k c", k=KT_D))

        _load_xt(0, nc.sync)
        _load_w("wq", nc.scalar)
        params_sb = singles.tile([P, 8], F32)
        nc.gpsimd.dma_start(params_sb, t["params"])
        bvr_sb = singles.tile([1, PAIRS * D2], F16)
        nc.gpsimd.dma_start(bvr_sb, t["bvr"])
        _load_w("wv", nc.gpsimd)
        _load_xt(1, nc.scalar)
        _load_w("wk", nc.sync)
        wo_sb = singles.tile([P, PAIRS, DIM], F16)
        nc.gpsimd.dma_start(wo_sb, t["wo"].rearrange("p (u c) -> p u c", u=PAIRS))
        _load_xt(2, nc.gpsimd)
        _load_xt(3, nc.sync)
        # bv broadcast across partitions once at setup; added during the
        # V^T psum eviction (varies along the free dim there)
        bvb_sb = singles.tile([P, PAIRS, D2], F16)
        nc.gpsimd.partition_broadcast(
            bvb_sb.rearrange("p a b -> p (a b)"), bvr_sb, channels=P
        )
        ident = singles.tile([P, P], F16)
        make_identity(nc, ident)
        # V^T for both pairs, with a ones column at 128 of each VW slot
        # (the PV moving operand is 129 wide; col 128 yields the softmax
        # denominator in Up[:, k, 128]).
        V_sb = singles.tile([P, MT, PAIRS, VW], F16)
        nc.vector.memset(V_sb[:, :, :, D2 : D2 + 1], 1.0)
        # transposed normalized outputs, [vd, pair, n]
        outnT = singles.tile([P, PAIRS, N], F16)

        env = dict(locals())
        loop_cm = (
            tc.For_i(0, iters, 1, hint_engines=(mybir.EngineType.PE,))
            if iters > 1
            else contextlib.nullcontext()
        )
        with loop_cm:
            _emit_body(tc, t, env)


def _emit_body(tc, t, env):
    nc = tc.nc
    Act = mybir.ActivationFunctionType
    Alu = mybir.AluOpType
    big = env["big"]; expp = env["expp"]; udp = env["udp"]; onp = env["onp"]
    sm = env["sm"]; ypool = env["ypool"]
    pa = env["pa"]; pu = env["pu"]; pb = env["pb"]
    xt_sb = env["xt_sb"]; w_sb = env["w_sb"]; wo_sb = env["wo_sb"]
    params_sb = env["params_sb"]; bvb_sb = env["bvb_sb"]
    ident = env["ident"]; V_sb = env["V_sb"]; outnT = env["outnT"]

    # ---- phase 1: QT,KT [d-pair, n] per pair; V^T [token, pair, vd] ----
    # V^T matmul groups (single-buffered pb psum) are interleaved between
    # the Q/K accumulation groups so their DVE evictions hide under PE work.
    QTs, KTs = [], []
    for u in range(PAIRS):
        QT = big.tile([P, N], F16, tag=f"QT{u}", name=f"QT{u}")
        KTt = big.tile([P, N], F16, tag=f"KT{u}", name=f"KT{u}")
        QTs.append(QT)
        KTs.append(KTt)

    def _emit_vt(nch, i):
        tt = nch * (CH // P) + i
        xc = xt_sb[nch]
        vt = pb.tile([P, PAIRS, D2], F32, tag="pb", name="vt")
        for kt in range(KT_D):
            nc.tensor.matmul(
                vt.rearrange("p a b -> p (a b)"),
                lhsT=xc[:, kt, i * P : (i + 1) * P],
                rhs=w_sb["wv"][:, kt, :],
                start=(kt == 0),
                stop=(kt == KT_D - 1),
            )
        nc.vector.tensor_tensor(V_sb[:, tt, :, 0:D2], vt, bvb_sb, Alu.add)

    # emission order per chunk: Q(p0), Q(p1), V^T x4, K(p0), K(p1) — the K
    # groups come ~5us into the chunk, relaxing the wk DMA arrival deadline
    # so the three input rings all land just in time
    for nch in range(NCH):
        sl = slice(nch * CH, (nch + 1) * CH)
        xc = xt_sb[nch]
        pss = []
        for u in range(PAIRS):
            usl = slice(u * D2, (u + 1) * D2)
            ps = pa.tile([P, 2, CH], F32, tag="pa")
            pss.append(ps)
            for kt in range(KT_D):
                nc.tensor.matmul(
                    ps[:, 0, :],
                    lhsT=w_sb["wq"][:, kt, usl],
                    rhs=xc[:, kt, :],
                    start=(kt == 0),
                    stop=(kt == KT_D - 1),
                )
            nc.vector.tensor_scalar_add(
                QTs[u][:, sl], ps[:, 0, :], params_sb[:, u : u + 1]
            )
        for i in range(CH // P):
            _emit_vt(nch, i)
        for u in range(PAIRS):
            usl = slice(u * D2, (u + 1) * D2)
            ps = pss[u]
            for kt in range(KT_D):
                nc.tensor.matmul(
                    ps[:, 1, :],
                    lhsT=w_sb["wk"][:, kt, usl],
                    rhs=xc[:, kt, :],
                    start=(kt == 0),
                    stop=(kt == KT_D - 1),
                )
            nc.vector.tensor_scalar_add(
                KTs[u][:, sl], ps[:, 1, :], params_sb[:, 2 + u : 3 + u]
            )

    if DEBUG:
        nc.gpsimd.dma_start(t["dbg_qt"], QTs[0])
        nc.gpsimd.dma_start(t["dbg_kt"], KTs[0])
        nc.gpsimd.dma_start(
            t["dbg_v"], V_sb[:, :, 0, 0:D2].rearrange("p a b -> p (a b)")
        )

    # Up accumulator layout: k = 4*h + j -> tile k//3, slot k%3.
    # Each slot is 129 f32 (128 vd + den), strided US to stay in-bank.
    def _up_slot(ups, k):
        return ups[k // 3][:, k % 3, 0:129]

    # ---- phase 2 (both pairs): S^T -> exp -> U[q,vd]+den via ones col ----
    # Each pair's onx-scale + transpose work is DEFERRED: emitted one chunk
    # at a time inside the next pair's chunk loop (pair 1: inside phase 3),
    # so the PE transposes interleave with S/PV matmuls instead of idling
    # behind the rinv(Act)/onx(DVE) chain.
    deferred = []

    def _emit_out(u, nch, ud, rinv):
        def emit():
            onx = onp.tile([P, 4, P], F16, tag="on", name="on")
            for j in range(4):
                nc.vector.tensor_scalar_mul(
                    onx[:, j, :], ud[:, j, :],
                    rinv[:, nch * 4 + j : nch * 4 + j + 1],
                )
            for j in range(4):
                # one transpose per psum tile: f16 transpose output cannot
                # share a psum accumulation group (HW faults).  (XBAR DMA
                # transpose measured 20us slower end-to-end — reverted.)
                pst = pb.tile([P, P], F16, tag="pb", name="pst")
                nc.tensor.transpose(pst, onx[:, j, :], ident)
                nc.vector.tensor_copy(
                    outnT[:, u, nch * CH + j * P : nch * CH + (j + 1) * P],
                    pst,
                )
        return emit

    for u in range(PAIRS):
        QT, KTt = QTs[u], KTs[u]
        uds = []
        msq = sm.tile([P, MT], F32, tag="msq", bufs=2)
        epsq_all = sm.tile([P, MT], F32, tag="epsq_all", bufs=2)
        rinv = sm.tile([P, MT], F32, tag="rinv", bufs=2)
        for nch in range(NCH):
            # PE queue is strict FIFO: only emit deferred transposes where
            # their rinv chain (act) is guaranteed drained — pair-0's items
            # pop inside pair-1's later chunks, pair-1's inside phase 3.
            if u == 1 and nch >= 1:
                deferred.pop(0)()
                if nch <= 2:
                    deferred.pop(0)()
            sl = slice(nch * CH, (nch + 1) * CH)
            ups = [
                pu.tile([P, 3, US], F32, tag="upA", name="upA"),
                pu.tile([P, 3, US], F32, tag="upB", name="upB"),
                pu.tile([P, 3, US], F32, tag="upC", name="upC"),
            ]

            exq = {}

            def _issue_exp(mt, sp):
                ex = expp.tile([P, 2, CH], F16, tag="ex", name="ex")
                # All exp on the Activation engine, one full [P,2,512] tile
                # per instr.  Measured dead ends: DVE custom-op exp (2.4us/
                # tile, FIFO stalls), strided act split (819ns for 768 elems
                # — worse than 1125ns for 1024).
                nc.scalar.activation(ex, sp, Act.Exp)
                exq[mt] = ex

            def _issue_pv(mt):
                ex = exq.pop(mt)
                for h in range(2):
                    for j in range(CH // P):
                        lh = ex[:, h, j * P : (j + 1) * P]
                        k = 4 * h + j
                        # one start/stop per psum tile (zero region):
                        # first/last k emitted into tile k//3 is k%3==0 /
                        # k%3==2 (tile 2 ends at k=7, k%3==1).
                        nc.tensor.matmul(
                            _up_slot(ups, k),
                            lhsT=lh,
                            rhs=V_sb[:, mt, u, 0 : D2 + 1],
                            start=(mt == 0 and k % 3 == 0),
                            stop=(mt == MT - 1 and (k % 3 == 2 or k == 7)),
                        )

            for mt in range(MT):
                msl = slice(mt * P, (mt + 1) * P)
                sp = pa.tile([P, 2, CH], F32, tag="pa")
                nc.tensor.matmul(
                    sp[:, 0, :], lhsT=KTt[0:64, msl], rhs=QT[0:64, sl]
                )
                nc.tensor.matmul(
                    sp[:, 1, :], lhsT=KTt[64:128, msl], rhs=QT[64:128, sl]
                )
                _issue_exp(mt, sp)
                if mt >= 2:
                    _issue_pv(mt - 2)
            _issue_pv(MT - 2)
            _issue_pv(MT - 1)

            # ---- post-processing: all per-partition [128, 4] ops ----
            # order: Up readers (ud) first so the single-buffered pu frees
            # for the next chunk's accumulation as early as possible.
            # DVE may read only ONE non-scalar input from PSUM per op:
            # stage s1 and U1 to SBUF first.
            # s1 = den of k0..3 = A0,A1,A2,B0 @ col 128; s2 = B1,B2,C0,C1.
            s1c = sm.tile([P, 4], F32, tag="s1c")
            nc.vector.tensor_copy(s1c[:, 0:3], ups[0][:, :, 128])
            nc.vector.tensor_copy(s1c[:, 3:4], ups[1][:, 0:1, 128])
            r2 = sm.tile([P, 4], F32, tag="r2")
            nc.vector.reciprocal(r2[:, 0:2], ups[1][:, 1:3, 128])
            nc.vector.reciprocal(r2[:, 2:4], ups[2][:, 0:2, 128])
            # f = -lam * s1 / s2   (params[:,7] = -lam)
            f = sm.tile([P, 4], F32, tag="f")
            nc.vector.scalar_tensor_tensor(
                out=f, in0=s1c, scalar=params_sb[:, 7:8],
                in1=r2, op0=Alu.mult, op1=Alu.mult,
            )
            u1c = udp.tile([P, 4, P], F32, tag="u1c")
            nc.vector.tensor_copy(u1c[:, 0:3, :], ups[0][:, :, 0:128])
            nc.vector.tensor_copy(u1c[:, 3, :], ups[1][:, 0, 0:128])
            ud = udp.tile([P, 4, P], F16, tag="ud", bufs=5)
            for j in range(4):
                # ud = U1 + f*U2  (f negated)
                nc.vector.scalar_tensor_tensor(
                    out=ud[:, j, :], in0=_up_slot(ups, 4 + j)[:, 0:128],
                    scalar=f[:, j : j + 1], in1=u1c[:, j, :],
                    op0=Alu.mult, op1=Alu.add,
                )
            # epsq = (P*eps) * s1^2  (added to ssq per pair, before Ln)
            nc.vector.scalar_tensor_tensor(
                out=epsq_all[:, nch * 4 : nch * 4 + 4], in0=s1c,
                scalar=float(P) * EPS,
                in1=s1c, op0=Alu.mult, op1=Alu.mult,
            )
            # ssq: square (f16 fast) + free-axis reduce; tensor_tensor_reduce
            # faults TRN2 here, so use the two-op form
            sqscr = udp.tile([P, 4, P], F32, tag="sq")
            nc.vector.tensor_mul(sqscr, ud, ud)
            uds.append(ud)
            for j in range(4):
                nc.vector.reduce_sum(
                    out=msq[:, nch * 4 + j : nch * 4 + j + 1],
                    in_=sqscr[:, j, :],
                    axis=mybir.AxisListType.X,
                )
            # rinv = rsqrt((ssq + P*eps*s1^2)/P) = exp(-0.5*ln(msqe/P)),
            # batched Ln/Exp (per-chunk [P,4] versions cost 790ns each).
            # Pair 1 computes rinv in two halves so its first two chunks'
            # transposes can pop before phase 3 with the act FIFO drained.
            if u == 1 and nch in (1, 3):
                hsl = slice((nch - 1) * 4, (nch + 1) * 4)
                nc.vector.tensor_add(
                    msq[:, hsl], msq[:, hsl], epsq_all[:, hsl]
                )
                lnmh = sm.tile([P, 8], F32, tag="lnmh", bufs=2)
                nc.scalar.activation(lnmh, msq[:, hsl], Act.Ln, scale=1.0 / P)
                nc.scalar.activation(rinv[:, hsl], lnmh, Act.Exp, scale=-0.5)
                deferred.append(_emit_out(u, nch - 1, uds[nch - 1], rinv))
                deferred.append(_emit_out(u, nch, uds[nch], rinv))

        if u == 0:
            nc.vector.tensor_add(msq, msq, epsq_all)
            lnm = sm.tile([P, MT], F32, tag="lnm", bufs=2)
            nc.scalar.activation(lnm, msq, Act.Ln, scale=1.0 / P)
            nc.scalar.activation(rinv, lnm, Act.Exp, scale=-0.5)
            for nch in range(NCH):
                deferred.append(_emit_out(u, nch, uds[nch], rinv))

    # ---- phase 3: y[q, d] = sum_u outnT_u^T @ wo_u ----
    # one [128, 1024] row-tile per q-tile through the freed pa slots (2
    # banks each); full contiguous DMA lines out.  Pair 1's deferred
    # onx/transpose for chunk c is emitted just before chunk c's q-tiles.
    for qt in range(MT):
        # remaining deferred outputs (pair-0 chunk 3 + all of pair-1):
        # p1 chunk c's transposes must land before qt==4c; qt 0 takes two
        if qt in (0, 4, 8) and deferred:
            deferred.pop(0)()  # p1c1 @ qt0, p1c2 @ qt4, p1c3 @ qt8
        qsl = slice(qt * P, (qt + 1) * P)
        yp = pa.tile([P, 2, 512], F32, tag="pa", name="y")
        for dc in range(2):
            dsl = slice(dc * 512, (dc + 1) * 512)
            for u in range(PAIRS):
                nc.tensor.matmul(
                    yp[:, dc, :],
                    lhsT=outnT[:, u, qsl],
                    rhs=wo_sb[:, u, dsl],
                    start=(u == 0),
                    stop=(u == PAIRS - 1),
                )
        ysb = ypool.tile([P, 2, 512], F16, tag="ysb")
        # split the eviction across both engines so the pa slot frees fast
        nc.vector.tensor_copy(ysb[:, 0, :], yp[:, 0, :])
        nc.scalar.activation(ysb[:, 1, :], yp[:, 1, :], Act.Identity)
        # gpsimd SWDGE enqueue is async (~430ns engine time) vs the sync
        # HWDGE dma_start which blocks the queue for the HBM-write latency
        nc.gpsimd.dma_start(
            t["ypart"][qsl, :].rearrange("p (a b) -> p a b", a=2), ysb
        )

    if DEBUG:
        nc.gpsimd.dma_start(t["dbg_onT"], outnT[:, 0, :])
        nc.gpsimd.dma_start(t["dbg_onT1"], outnT[:, 1, :])


def build_program(iters=1):
    nc = bacc.Bacc("TRN2", target_bir_lowering=False, debug=False)
    t = {
        # host pre-arranged: xt[n*128+p, k*512+c]; w*[p, k*256+c]; wo[p, u*1024+c]
        "xt": nc.dram_tensor("xt", [NCH * P, KT_D * CH], F16, kind="ExternalInput")[:],
        "wq": nc.dram_tensor("wq", [P, KT_D * PAIRS * D2], F16, kind="ExternalInput")[:],
        "wk": nc.dram_tensor("wk", [P, KT_D * PAIRS * D2], F16, kind="ExternalInput")[:],
        "wv": nc.dram_tensor("wv", [P, KT_D * PAIRS * D2], F16, kind="ExternalInput")[:],
        "wo": nc.dram_tensor("wo", [P, PAIRS * DIM], F16, kind="ExternalInput")[:],
        "params": nc.dram_tensor("params", [P, 8], F32, kind="ExternalInput")[:],
        "bvr": nc.dram_tensor("bvr", [1, PAIRS * D2], F16, kind="ExternalInput")[:],
        "ypart": nc.dram_tensor("ypart", [N, DIM], F16, kind="ExternalOutput")[:],
    }
    if DEBUG:
        for nm, shp in [
            ("dbg_qt", [P, N]), ("dbg_kt", [P, N]), ("dbg_v", [P, MT * P]),
            ("dbg_rinv", [P, 16]),
            ("dbg_onT", [P, N]), ("dbg_onT1", [P, N]),
        ]:
            t[nm] = nc.dram_tensor(nm, shp, F32, kind="ExternalOutput")[:]
    with tile.TileContext(nc) as tc:
        _emit(tc, t, iters=iters)
    nc.compile()
    return nc


_NC_CACHE = {}


def _get_nc(iters=1):
    if iters not in _NC_CACHE:
        _NC_CACHE[iters] = build_program(iters)
    return _NC_CACHE[iters]


def make_core_inputs(x, Wq, bq, Wk, bk, Wv, bv, Wo, bo, g, lq1, lk1, lq2, lk2):
    """Host-side shard prep.  Returns (in_maps, lam) for the 8 cores."""
    x = np.asarray(x, np.float32)
    scaling = np.float32(HEAD_DIM**-0.5)
    lam1 = np.exp(np.sum(np.asarray(lq1, np.float32) * np.asarray(lk1, np.float32)))
    lam2 = np.exp(np.sum(np.asarray(lq2, np.float32) * np.asarray(lk2, np.float32)))
    lam = np.float32(lam1 - lam2 + LAMBDA_INIT)

    xt = np.ascontiguousarray(np.transpose(x, (0, 2, 1)))  # (B, DIM, N)
    Wq_s = np.asarray(Wq, np.float32) * scaling
    bq_s = np.asarray(bq, np.float32) * scaling
    geff = (np.asarray(g, np.float32) * np.float32(1.0 - LAMBDA_INIT)).reshape(P)

    in_maps = []
    for c in range(CORES):
        b = c // 4
        grp = c % 4
        cols = slice(grp * PAIRS * D2, (grp + 1) * PAIRS * D2)
        params = np.zeros((P, 8), np.float32)
        params[:, 0:2] = bq_s[cols].reshape(PAIRS, P).T
        params[:, 2:4] = np.asarray(bk, np.float32)[cols].reshape(PAIRS, P).T
        params[:, 7] = -lam
        # Wo rows for this core, pre-scaled by g*(1-lam_init) per vd
        wo_eff = np.asarray(Wo, np.float32)[cols, :] * np.tile(geff, PAIRS)[:, None]

        def _wlay(W):  # [DIM, C] -> [P, KT_D*C]: w[p, k*C+c] = W[k*128+p, c]
            C = W.shape[1]
            return np.ascontiguousarray(
                W.reshape(KT_D, P, C).transpose(1, 0, 2).reshape(P, KT_D * C)
            )

        # xt chunk-contiguous: xtc[n*128+p, k*512+c] = xt[k*128+p, n*512+c]
        xtc = (
            xt[b]
            .reshape(KT_D, P, NCH, CH)
            .transpose(2, 1, 0, 3)
            .reshape(NCH * P, KT_D * CH)
        )
        in_maps.append(
            {
                "xt": np.ascontiguousarray(xtc).astype(np.float16),
                "wq": _wlay(Wq_s[:, cols]).astype(np.float16),
                "wk": _wlay(np.asarray(Wk, np.float32)[:, cols]).astype(np.float16),
                "wv": _wlay(np.asarray(Wv, np.float32)[:, cols]).astype(np.float16),
                "wo": np.ascontiguousarray(
                    wo_eff.reshape(PAIRS, P, DIM)
                    .transpose(1, 0, 2)
                    .reshape(P, PAIRS * DIM)
                ).astype(np.float16),
                "params": params,
                "bvr": np.asarray(bv, np.float32)[cols]
                .reshape(1, PAIRS * D2)
                .astype(np.float16),
            }
        )
    return in_maps, lam


def gather_output(results, bo):
    """Sum per-core y partials per batch, add bo."""
    bo = np.asarray(bo, np.float32)
    out = np.empty((B, N, DIM), np.float32)
    for b in range(B):
        acc = np.zeros((N, DIM), np.float32)
        for c in range(b * 4, b * 4 + 4):
            acc += results[c]["ypart"].astype(np.float32)
        out[b] = acc + bo
    return out


_IN_CACHE = {}


def kernel(**inputs):
    from concourse.bass_utils import run_bass_kernel_spmd

    key = id(inputs.get("x"))
    if key in _IN_CACHE:
        in_maps = _IN_CACHE[key]
    else:
        in_maps, _ = make_core_inputs(**inputs)
        _IN_CACHE.clear()
        _IN_CACHE[key] = in_maps
    iters = int(os.environ.get("KERNEL_ITERS", "1"))
    nc = _get_nc(iters)
    trace = bool(int(os.environ.get("KERNEL_TRACE", "0")))
    res = run_bass_kernel_spmd(
        nc, in_maps, core_ids=list(range(CORES)), trace=trace
    )
    if trace and res.exec_time_ns is not None:
        print(f"HW exec time: {res.exec_time_ns} ns")
        kernel.last_exec_time_ns = res.exec_time_ns
        kernel.last_trace = res.instructions_and_trace
    return gather_output(res.results, inputs["bo"])


# ---------------- dev helpers (not used by the grading harness) ----------------


def _numpy_core_partial(im):
    """Reference computation of one core's ypart from its sharded inputs."""
    # undo the host DMA layouts
    xt = (
        im["xt"]
        .reshape(NCH, P, KT_D, CH)
        .transpose(2, 1, 0, 3)
        .reshape(DIM, N)
        .astype(np.float64)
    )
    wq = im["wq"].reshape(P, KT_D, -1).transpose(1, 0, 2).reshape(DIM, -1)
    wk = im["wk"].reshape(P, KT_D, -1).transpose(1, 0, 2).reshape(DIM, -1)
    wv = im["wv"].reshape(P, KT_D, -1).transpose(1, 0, 2).reshape(DIM, -1)
    wo = im["wo"].reshape(P, PAIRS, DIM).transpose(1, 0, 2).reshape(PAIRS * D2, DIM)
    x = xt.T
    pr = im["params"]
    lam = -float(pr[0, 7])
    ypart = np.zeros((N, DIM))
    for u in range(PAIRS):
        usl = slice(u * D2, (u + 1) * D2)
        q = x @ wq[:, usl].astype(np.float64) + pr[:, u]  # [N, 128]
        k = x @ wk[:, usl].astype(np.float64) + pr[:, 2 + u]
        v = x @ wv[:, usl].astype(np.float64) + im["bvr"][0, usl].astype(
            np.float64
        )
        s1 = q[:, :64] @ k[:, :64].T
        s2 = q[:, 64:] @ k[:, 64:].T
        p1 = np.exp(s1)
        p1 /= p1.sum(-1, keepdims=True)
        p2 = np.exp(s2)
        p2 /= p2.sum(-1, keepdims=True)
        diff = p1 - lam * p2
        o = diff @ v  # [N, 128]
        rms = 1.0 / np.sqrt((o * o).mean(-1, keepdims=True) + EPS)
        o = o * rms
        ypart += o @ wo[usl, :].astype(np.float64)
    return ypart


if __name__ == "__main__":
    mode = sys.argv[1] if len(sys.argv) > 1 else "sim"
    sys.path.insert(0, "/root/problem")
    import reference

    inputs = {k: np.asarray(v) for k, v in reference.setup_inputs().items()}
    in_maps, lam = make_core_inputs(**inputs)
    print("lam =", lam)
    nc = _get_nc()
    if mode == "sim":
        from concourse.bass_interp import CoreSim

        sim = CoreSim(nc)
        for k, v in in_maps[0].items():
            sim.tensor(k)[:] = v
        sim.simulate()
        got = np.array(sim.tensor("ypart")).astype(np.float64)
        want = _numpy_core_partial(in_maps[0])
        err = np.abs(got - want)
        scale = np.abs(want).max()
        print("absmax err:", err.max(), "rel:", err.max() / scale, "scale:", scale)
        try:
            print("sim predicted time:", sim.time, "ns")
        except Exception as e:
            print("no sim time:", e)
